# revision 1
# baseline (speedup 1.0000x reference)
"""Bayesian GPLVM collapsed-ELBO kernel for Trainium2 (8 NeuronCores).

Sharding: data-parallel over n (2048 rows -> 256 per core). Each core
computes its partial psi2 = sum_n exp(log_psi2_n) (m*m = 4096 entries),
partial A = psi1^T y (64x256), and partial row statistics (KL pieces,
sum y^2). Host sums the 8 partials and does the small m x m linear
algebra (Cholesky solves / slogdets) to produce the scalar ELBO.

Device layout per core (n_loc = 256, two 128-row chunks):
  - NPREP (98 x 256, q-major n-side): rows 0..15 = (q_mu*w1)^T,
    16..31 = w1^T, 32 = h1, 64..79 = (q_mu*w2)^T, 80..95 = w2^T,
    96 = g, 97 = ones (matmul operands need base partition in
    {0,32,64}, so the psi2 block sits at 64). Built n-major as a
    (128 x 98) tile per chunk, then PE-transposed.
  - psi1 exponent = NPREP[0:33,chunk]^T @ ZS1 (z-side, host-built),
    one matmul + Exp per chunk; A accumulates psi1^T y in PSUM.
  - psi2 exponent for each of 32 ij-chunks (128 ij-pairs each) =
    ZL[64:98, chunk]^T @ NPREP[64:98]; Exp with fused free-axis
    accumulation gives the local n-sum directly.
"""

import numpy as np

N, D, Q, M = 2048, 256, 16, 64
NCORES = 8
NLOC = N // NCORES          # 256
F32 = None                  # set lazily (mybir.dt.float32)

_compiled = None


def _build_bass():
    import concourse.bacc as bacc
    import concourse.bass as bass  # noqa: F401
    import concourse.mybir as mybir
    from concourse import masks
    from concourse.tile import TileContext

    f32 = mybir.dt.float32
    f32r = mybir.dt.float32r
    AF = mybir.ActivationFunctionType
    OP = mybir.AluOpType

    nc = bacc.Bacc("TRN2", target_bir_lowering=False, num_swdge_queues=2)

    y_d = nc.declare_dram_parameter("y", [NLOC, D], f32r, isOutput=False)
    qin_d = nc.declare_dram_parameter("qin", [NLOC, 2 * Q], f32, isOutput=False)
    zl_d = nc.declare_dram_parameter("zl", [34, 17 * 128], f32r, isOutput=False)
    zs1_d = nc.declare_dram_parameter("zs1", [33, M], f32r, isOutput=False)
    acon_d = nc.declare_dram_parameter("acon", [128, Q + 4], f32, isOutput=False)
    psi2_o = nc.declare_dram_parameter("out_psi2", [128, 17], f32, isOutput=True)
    a_o = nc.declare_dram_parameter("out_A", [M, D], f32, isOutput=True)
    misc_o = nc.declare_dram_parameter("out_misc", [128, 8], f32, isOutput=True)

    with TileContext(nc) as tc:
        with (
            tc.tile_pool(name="const", bufs=1) as cpool,
            tc.tile_pool(name="big", bufs=1) as bigpool,
            tc.tile_pool(name="work", bufs=3) as wpool,
            tc.tile_pool(name="scr", bufs=3) as spool,
            tc.tile_pool(name="psum", bufs=2, space="PSUM") as ppool,
            tc.tile_pool(name="psums", bufs=1, space="PSUM") as ppools,
            tc.tile_pool(name="psum1", bufs=1, space="PSUM") as ppool1,
        ):
            ident = cpool.tile([128, 128], f32)
            masks.make_identity(nc, ident[:])

            acon = cpool.tile([128, Q + 4], f32)
            nc.sync.dma_start(out=acon[:, :], in_=acon_d[:, :])
            alpha_b = acon[:, 0:Q]
            consts_b = acon[:, Q:Q + 4]

            zl_sb = bigpool.tile([98, 17 * 128], f32r)
            zs1_sb = cpool.tile([33, M], f32r)

            nprep = bigpool.tile([98, 2 * 128], f32r)
            stats = bigpool.tile([128, 17], f32)
            misc = bigpool.tile([128, 8], f32)
            apsum = ppool1.tile([M, D], f32)

            zs1_loaded = False
            for c in range(2):
                r0, r1 = c * 128, (c + 1) * 128
                qin = wpool.tile([128, 2 * Q], f32, tag="qin")
                nc.sync.dma_start(out=qin[:, :], in_=qin_d[r0:r1, :])
                qm = qin[:, 0:Q]
                qls = qin[:, Q:2 * Q]
                yc = wpool.tile([128, D], f32r, tag="yc")
                nc.sync.dma_start(out=yc[:, :], in_=y_d[r0:r1, :])
                if not zs1_loaded:
                    nc.sync.dma_start(out=zs1_sb[:, :], in_=zs1_d[:, :])
                    zs1_loaded = True

                prepn = wpool.tile([128, 98], f32, tag="prepn")
                qsig = wpool.tile([128, Q], f32, tag="qsig")
                d1 = wpool.tile([128, Q], f32, tag="d1")
                d2 = wpool.tile([128, Q], f32, tag="d2")
                rcp = wpool.tile([128, Q], f32, tag="rcp")
                scr16 = spool.tile([128, Q], f32, tag="scr16")
                scrY = spool.tile([128, D], f32, tag="scrY")
                cols = wpool.tile([128, 8], f32, tag="cols")
                sum2c = cols[:, 0:1]
                s3x2c = cols[:, 1:2]
                rt1c = cols[:, 2:3]
                ac = cols[:, 3:4]
                t1c = cols[:, 4:5]
                t2c = cols[:, 5:6]

                # q_sigma = softplus(qls) = ln(1 + exp(qls))
                nc.scalar.activation(scr16[:, :], qls, AF.Exp)
                nc.scalar.activation(qsig[:, :], scr16[:, :], AF.Ln, bias=1.0)
                nc.vector.tensor_mul(d1[:, :], qsig[:, :], alpha_b)
                nc.vector.tensor_scalar_add(d1[:, :], d1[:, :], 1.0)
                # w1 = alpha / d1
                nc.vector.reciprocal(rcp[:, :], d1[:, :])
                nc.vector.tensor_mul(prepn[:, 16:32], rcp[:, :], alpha_b)
                # sum2 = sum_q log d1
                nc.scalar.activation(scr16[:, :], d1[:, :], AF.Ln, accum_out=sum2c)
                # d2 = 2*d1 - 1;  w2 = alpha / d2
                nc.vector.tensor_scalar(
                    out=d2[:, :], in0=d1[:, :], scalar1=2.0, scalar2=-1.0,
                    op0=OP.mult, op1=OP.add)
                nc.vector.reciprocal(rcp[:, :], d2[:, :])
                nc.vector.tensor_mul(prepn[:, 80:96], rcp[:, :], alpha_b)
                # 2*s3 = sum_q log d2
                nc.scalar.activation(scr16[:, :], d2[:, :], AF.Ln, accum_out=s3x2c)
                # q_mu * w1, q_mu * w2
                nc.vector.tensor_mul(prepn[:, 0:16], qm, prepn[:, 16:32])
                nc.vector.tensor_mul(prepn[:, 64:80], qm, prepn[:, 80:96])
                # rt1 = sum_q q_mu^2 w1 ; a = sum_q q_mu^2 w2
                nc.vector.tensor_mul(scr16[:, :], prepn[:, 0:16], qm)
                nc.vector.tensor_reduce(rt1c, scr16[:, :],
                                        axis=mybir.AxisListType.X, op=OP.add)
                nc.vector.tensor_mul(scr16[:, :], prepn[:, 64:80], qm)
                nc.vector.tensor_reduce(ac, scr16[:, :],
                                        axis=mybir.AxisListType.X, op=OP.add)
                # h1 = 2*logvar - 0.5*(rt1 + sum2)
                nc.vector.tensor_add(t1c, rt1c, sum2c)
                nc.vector.tensor_scalar(
                    out=prepn[:, 32:33], in0=t1c, scalar1=-0.5,
                    scalar2=consts_b[:, 0:1], op0=OP.mult, op1=OP.add)
                # g = 4*logvar - 0.5*(2*s3) - a
                nc.vector.tensor_scalar(
                    out=t2c, in0=s3x2c, scalar1=0.5, scalar2=ac,
                    op0=OP.mult, op1=OP.add)
                nc.vector.tensor_scalar(
                    out=prepn[:, 96:97], in0=t2c, scalar1=-1.0,
                    scalar2=consts_b[:, 1:2], op0=OP.mult, op1=OP.add)
                nc.vector.memset(prepn[:, 97:98], 1.0)
                nc.vector.memset(prepn[:, 33:64], 0.0)

                # KL / trace statistics (squares on DVE, Ln stays on ACT)
                nc.scalar.activation(scr16[:, :], qsig[:, :], AF.Ln,
                                     accum_out=misc[:, 0 + c:1 + c])
                nc.vector.tensor_mul(scr16[:, :], qsig[:, :], qsig[:, :])
                nc.vector.tensor_reduce(misc[:, 2 + c:3 + c], scr16[:, :],
                                        axis=mybir.AxisListType.X, op=OP.add)
                nc.vector.tensor_mul(scr16[:, :], qm, qm)
                nc.vector.tensor_reduce(misc[:, 4 + c:5 + c], scr16[:, :],
                                        axis=mybir.AxisListType.X, op=OP.add)
                nc.vector.tensor_mul(scrY[:, :], yc[:, :].bitcast(f32), yc[:, :].bitcast(f32))
                nc.vector.tensor_reduce(misc[:, 6 + c:7 + c], scrY[:, :],
                                        axis=mybir.AxisListType.X, op=OP.add)

                # transpose prep (128 x 67) -> NPREP[:, chunk]
                ptp = ppools.tile([98, 128], f32, tag="ptp")
                nc.tensor.transpose(ptp[:, :], prepn[:, :], ident[:, :])
                nc.vector.tensor_copy(nprep[:, r0:r1], ptp[:, :])

                # psi1 chunk: exponent (128 n x 64 m) then exp
                e1 = ppools.tile([128, M], f32, tag="e1")
                nc.tensor.matmul(e1[:, :],
                                 lhsT=nprep[0:33, r0:r1],
                                 rhs=zs1_sb[:, :],
                                 start=True, stop=True)
                psi1c = wpool.tile([128, M], f32r, tag="psi1c")
                nc.scalar.activation(psi1c[:, :], e1[:, :], AF.Exp)
                # A += psi1_c^T @ y_c
                nc.tensor.matmul(apsum[:, :], lhsT=psi1c[:, :],
                                 rhs=yc[:, :],
                                 start=(c == 0), stop=(c == 1))

            # zl is big (~290KB): issue after the small DMAs in sync-engine
            # program order (ring completes in order), split per chunk-group
            # so each psi2 group starts as soon as its slice lands
            for t in range(5):
                c0, c1 = t * 512, min((t + 1) * 512, 17 * 128)
                nc.sync.dma_start(out=zl_sb[64:98, c0:c1], in_=zl_d[:, c0:c1])

            # psi2 is symmetric: only the 2080 upper-triangle ij-pairs
            # (17 chunks of 128, last 96 slots are padding), 4 chunks per
            # PSUM tile
            for t in range(5):
                nch = min(4, 17 - 4 * t)
                p2 = ppool.tile([128, 4 * NLOC], f32, tag="p2")
                for j in range(nch):
                    ch = 4 * t + j
                    nc.tensor.matmul(
                        p2[:, j * NLOC:(j + 1) * NLOC],
                        lhsT=zl_sb[64:98, ch * 128:(ch + 1) * 128],
                        rhs=nprep[64:98, :],
                        start=True, stop=True)
                scr = spool.tile([128, 4 * NLOC], f32, tag="p2scr")
                w = nch * NLOC
                nc.scalar.activation(scr[:, :w], p2[:, :w], AF.Exp)
                nc.vector.tensor_reduce(
                    stats[:, 4 * t:4 * t + nch],
                    scr[:, :w].rearrange("p (a b) -> p a b", b=NLOC),
                    axis=mybir.AxisListType.X, op=OP.add)

            a_sb = bigpool.tile([M, D], f32)
            nc.vector.tensor_copy(a_sb[:, :], apsum[:, :])
            nc.sync.dma_start(out=psi2_o[:, :], in_=stats[:, :])
            nc.sync.dma_start(out=a_o[:, :], in_=a_sb[:, :])
            nc.sync.dma_start(out=misc_o[:, :], in_=misc[:, :])

    nc.compile()
    return nc


def _get_compiled():
    global _compiled
    if _compiled is None:
        _compiled = _build_bass()
    return _compiled


def _np_softplus(x):
    return np.logaddexp(x, 0.0)


def kernel(y, q_mu, q_log_sigma, z, noise_raw, alpha, variance, _trace=False):
    from concourse.bass_utils import run_bass_kernel_spmd

    nc = _get_compiled()

    f8 = np.float64
    z64 = z.astype(f8)
    al = alpha.astype(f8)
    var = f8(variance[0])
    logvar = np.log(var)

    # z-side stationary blocks (host-built, replicated to all cores).
    # psi2 is symmetric in (i, j): ship only the 2080 upper-tri pairs.
    iu, ju = np.triu_indices(M)                             # (2080,)
    npairs = iu.shape[0]
    Su = z64[iu] + z64[ju]                                  # (2080, q)
    sqz = (z64[:, None, :] - z64[None, :, :]) ** 2          # (m, m, q)
    s1 = 0.25 * (sqz @ al)                                  # (m, m)
    zl = np.zeros((34, 17 * 128), np.float32)
    zl[0:16, :npairs] = Su.T
    zl[16:32, :npairs] = (-0.25 * Su * Su).T
    zl[32, :npairs] = 1.0
    zl[33, :npairs] = -s1[iu, ju]

    zt = z64.T                                              # (q, m)
    zs1 = np.empty((33, M), np.float32)
    zs1[0:16] = zt
    zs1[16:32] = -0.5 * zt * zt
    zs1[32] = 1.0

    acon = np.empty((128, Q + 4), np.float32)
    acon[:, 0:Q] = alpha.reshape(1, Q).astype(np.float32)
    acon[:, Q:Q + 4] = np.array([2.0 * logvar, 4.0 * logvar, 0.0, 0.0],
                                np.float32)
    qin_full = np.concatenate(
        [q_mu.astype(np.float32), q_log_sigma.astype(np.float32)], axis=1)

    in_maps = []
    for i in range(NCORES):
        sl = slice(i * NLOC, (i + 1) * NLOC)
        in_maps.append({
            "y": np.ascontiguousarray(y[sl], dtype=np.float32),
            "qin": np.ascontiguousarray(qin_full[sl]),
            "zl": zl,
            "zs1": zs1,
            "acon": acon,
        })

    br = run_bass_kernel_spmd(nc, in_maps, list(range(NCORES)), trace=_trace)
    res = br.results

    psi2_part = np.zeros((128, 17), f8)
    A = np.zeros((M, D), f8)
    misc = np.zeros(8, f8)
    for r in res:
        psi2_part += r["out_psi2"].astype(f8)
        A += r["out_A"].astype(f8)
        misc += r["out_misc"].astype(f8).sum(axis=0)

    flat = psi2_part.T.reshape(17 * 128)
    psi2 = np.empty((M, M), f8)
    psi2[iu, ju] = flat[:npairs]
    psi2[ju, iu] = flat[:npairs]
    lnsig = misc[0] + misc[1]
    ssq = misc[2] + misc[3]
    musq = misc[4] + misc[5]
    tr_yy = misc[6] + misc[7]

    kl_sum = -lnsig + 0.5 * (ssq + musq) - 0.5 * N * Q
    kl_term = kl_sum / (N * D)

    # small m x m algebra on host
    k_mm = var * np.exp(-0.5 * (sqz @ al))                  # (m, m)
    noise_var = _np_softplus(f8(noise_raw[0]))
    beta = 1.0 / noise_var
    psi0 = N * var

    cov1 = beta * psi2 + k_mm
    B = np.linalg.solve(cov1, A)
    tr_yWy = beta * tr_yy - np.sum(A * B)

    F = 0.5 * N * np.log(beta)
    F += 0.5 * np.linalg.slogdet(k_mm)[1]
    F -= 0.5 * N * np.log(np.pi)
    F -= 0.5 * np.linalg.slogdet(cov1)[1]
    F -= 0.5 * beta * psi0
    F += 0.5 * np.trace(np.linalg.solve(k_mm, psi2))
    F = (F * D - 0.5 * tr_yWy) / (N * D)

    out = F - kl_term
    result = np.asarray(out, dtype=np.float32)
    if _trace:
        return result, br
    return result



# revision 2
# speedup vs baseline: 1.2317x; 1.2317x over previous
"""Bayesian GPLVM collapsed-ELBO kernel for Trainium2 (8 NeuronCores).

Sharding: data-parallel over n (2048 rows -> 256 per core). All O(n*q)
row prep (softplus, d1/d2, w1/w2, log-sums, KL pieces) is done on host
in float64 and shipped as small per-core operand blocks; the device does
only the O(n*m) / O(n*m^2) work:

  - psi1 exponent = p1^T @ zs1 (two 33x128x64 matmuls into one PSUM
    tile), one Exp, then A += psi1^T y accumulated in PSUM.
  - psi2: for each of 17 ij-chunks (128 upper-triangle pairs each),
    exponent = zl_chunk^T @ p2 (34x128x256, bf16), Exp on ACT, free-axis
    n-sum on DVE. Host sums the 8 per-core partials and does the small
    m x m linear algebra (Cholesky solves / slogdets).

Only the Exp activation table is ever needed on device, so there is a
single ACT_TABLE_LOAD that overlaps the input DMAs.
"""

import numpy as np

N, D, Q, M = 2048, 256, 16, 64
NCORES = 8
NLOC = N // NCORES          # 256
NPAIRS = 2080               # upper-triangle pairs of 64x64
NCHUNK = 17                 # ceil(2080 / 128)
GROUPS = [(0, 6), (6, 6), (12, 4), (16, 1)]

_compiled = None


def _build_bass():
    import concourse.bacc as bacc
    import concourse.bass as bass  # noqa: F401
    import concourse.mybir as mybir
    from concourse.tile import TileContext

    f32 = mybir.dt.float32
    f32r = mybir.dt.float32r
    bf16 = mybir.dt.bfloat16
    AF = mybir.ActivationFunctionType
    OP = mybir.AluOpType

    nc = bacc.Bacc("TRN2", target_bir_lowering=False, num_swdge_queues=2)

    p1z_d = nc.declare_dram_parameter("p1z", [33, 320], f32r, isOutput=False)
    p2_d = nc.declare_dram_parameter("p2", [34, NLOC], bf16, isOutput=False)
    y_d = nc.declare_dram_parameter("yb", [128, 512], f32r, isOutput=False)
    zl_d = nc.declare_dram_parameter("zl", [34, NCHUNK * 128], bf16,
                                     isOutput=False)
    psi2_o = nc.declare_dram_parameter("out_psi2", [128, NCHUNK], f32,
                                       isOutput=True)
    a_o = nc.declare_dram_parameter("out_A", [M, D], f32, isOutput=True)

    with TileContext(nc) as tc:
        with (
            tc.tile_pool(name="const", bufs=1) as cpool,
            tc.tile_pool(name="scr", bufs=2) as spool,
            tc.tile_pool(name="psum", bufs=2, space="PSUM") as ppool,
            tc.tile_pool(name="psume", bufs=1, space="PSUM") as ppool_e,
            tc.tile_pool(name="psuma", bufs=1, space="PSUM") as ppool_a,
        ):
            p1z = cpool.tile([33, 320], f32r)
            p2 = cpool.tile([34, NLOC], bf16)
            yb = cpool.tile([128, 512], f32r)
            zl = cpool.tile([34, NCHUNK * 128], bf16)
            stats = cpool.tile([128, NCHUNK], f32)
            psi1c = cpool.tile([128, 128], f32r)
            a_sb = cpool.tile([M, D], f32)

            nc.sync.dma_start(out=p1z[:, :], in_=p1z_d[:, :])
            nc.sync.dma_start(out=p2[:, :], in_=p2_d[:, :])
            nc.sync.dma_start(out=yb[:, :], in_=y_d[:, :])
            nc.sync.dma_start(out=zl[:, 0:1024], in_=zl_d[:, 0:1024])
            nc.sync.dma_start(out=zl[:, 1024:2176], in_=zl_d[:, 1024:2176])

            # psi1 exponent for both 128-row chunks into one PSUM tile
            e1 = ppool_e.tile([128, 128], f32)
            for c in range(2):
                nc.tensor.matmul(e1[:, c * 64:(c + 1) * 64],
                                 lhsT=p1z[:, c * 128:(c + 1) * 128],
                                 rhs=p1z[:, 256:320],
                                 start=True, stop=True)
            nc.scalar.activation(psi1c[:, :], e1[:, :], AF.Exp)

            apsum = ppool_a.tile([M, D], f32)
            for c in range(2):
                nc.tensor.matmul(apsum[:, :],
                                 lhsT=psi1c[:, c * 64:(c + 1) * 64],
                                 rhs=yb[:, c * 256:(c + 1) * 256],
                                 start=(c == 0), stop=(c == 1))
            nc.vector.tensor_copy(a_sb[:, :], apsum[:, :])
            nc.sync.dma_start(out=a_o[:, :], in_=a_sb[:, :])

            # psi2: 17 ij-chunks, grouped so exp/reduce pipeline with the
            # matmuls; last group is a single chunk to keep the tail short
            for ch0, nch in GROUPS:
                p2p = ppool.tile([128, 6 * NLOC], f32, tag="p2p")
                for j in range(nch):
                    ch = ch0 + j
                    nc.tensor.matmul(
                        p2p[:, j * NLOC:(j + 1) * NLOC],
                        lhsT=zl[:, ch * 128:(ch + 1) * 128],
                        rhs=p2[:, :],
                        start=True, stop=True)
                scr = spool.tile([128, 6 * NLOC], f32, tag="scr")
                w = nch * NLOC
                nc.scalar.activation(scr[:, :w], p2p[:, :w], AF.Exp)
                nc.vector.tensor_reduce(
                    stats[:, ch0:ch0 + nch],
                    scr[:, :w].rearrange("p (a b) -> p a b", b=NLOC),
                    axis=mybir.AxisListType.X, op=OP.add)
                nc.sync.dma_start(out=psi2_o[:, ch0:ch0 + nch],
                                  in_=stats[:, ch0:ch0 + nch])

    nc.compile()
    return nc


def _get_compiled():
    global _compiled
    if _compiled is None:
        _compiled = _build_bass()
    return _compiled


def kernel(y, q_mu, q_log_sigma, z, noise_raw, alpha, variance, _trace=False):
    import ml_dtypes
    from concourse.bass_utils import run_bass_kernel_spmd

    nc = _get_compiled()

    f8 = np.float64
    qm = q_mu.astype(f8)
    qls = q_log_sigma.astype(f8)
    z64 = z.astype(f8)
    al = alpha.astype(f8)
    var = f8(variance[0])
    logvar = np.log(var)

    # ---- host row prep (O(n*q)) ----
    qsig = np.logaddexp(qls, 0.0)                           # softplus
    d1 = qsig * al + 1.0
    d2 = 2.0 * al * qsig + 1.0
    w1 = al / d1
    w2 = al / d2
    lse1 = np.sum(np.log(d1), axis=1)                       # (n,)
    lse2 = np.sum(np.log(d2), axis=1)
    rt1 = np.sum(qm * qm * w1, axis=1)
    rt2 = np.sum(qm * qm * w2, axis=1)
    h1 = 2.0 * logvar - 0.5 * (rt1 + lse1)
    g = 4.0 * logvar - rt2 - 0.5 * lse2

    kl_sum = np.sum(-np.log(qsig) + 0.5 * (qsig * qsig + qm * qm - 1.0))
    tr_yy = np.sum(y.astype(f8) ** 2)

    # ---- z-side blocks (replicated) ----
    iu, ju = np.triu_indices(M)                             # (2080,)
    Su = z64[iu] + z64[ju]                                  # (2080, q)
    sqz = (z64[:, None, :] - z64[None, :, :]) ** 2          # (m, m, q)
    s1 = 0.25 * (sqz @ al)                                  # (m, m)
    zl = np.zeros((34, NCHUNK * 128), np.float32)
    zl[0:16, :NPAIRS] = Su.T
    zl[16:32, :NPAIRS] = (-0.25 * Su * Su).T
    zl[32, :NPAIRS] = 1.0
    zl[33, :NPAIRS] = -s1[iu, ju]
    zl = zl.astype(ml_dtypes.bfloat16)

    zt = z64.T                                              # (q, m)

    in_maps = []
    for i in range(NCORES):
        sl = slice(i * NLOC, (i + 1) * NLOC)
        p1z = np.zeros((33, 320), np.float32)
        p1z[0:16, 0:NLOC] = (qm[sl] * w1[sl]).T
        p1z[16:32, 0:NLOC] = w1[sl].T
        p1z[32, 0:NLOC] = h1[sl]
        p1z[0:16, 256:320] = zt
        p1z[16:32, 256:320] = -0.5 * zt * zt
        p1z[32, 256:320] = 1.0

        p2 = np.empty((34, NLOC), np.float32)
        p2[0:16] = (qm[sl] * w2[sl]).T
        p2[16:32] = w2[sl].T
        p2[32] = g[sl]
        p2[33] = 1.0

        yb = np.ascontiguousarray(
            y[sl].astype(np.float32).reshape(2, 128, D)
            .transpose(1, 0, 2).reshape(128, 512))

        in_maps.append({
            "p1z": p1z,
            "p2": p2.astype(ml_dtypes.bfloat16),
            "yb": yb,
            "zl": zl,
        })

    br = run_bass_kernel_spmd(nc, in_maps, list(range(NCORES)), trace=_trace)
    res = br.results

    psi2_part = np.zeros((128, NCHUNK), f8)
    A = np.zeros((M, D), f8)
    for r in res:
        psi2_part += r["out_psi2"].astype(f8)
        A += r["out_A"].astype(f8)

    flat = psi2_part.T.reshape(NCHUNK * 128)
    psi2 = np.empty((M, M), f8)
    psi2[iu, ju] = flat[:NPAIRS]
    psi2[ju, iu] = flat[:NPAIRS]

    kl_term = kl_sum / (N * D)

    # small m x m algebra on host
    k_mm = var * np.exp(-0.5 * (sqz @ al))                  # (m, m)
    noise_var = np.logaddexp(f8(noise_raw[0]), 0.0)
    beta = 1.0 / noise_var
    psi0 = N * var

    cov1 = beta * psi2 + k_mm
    B = np.linalg.solve(cov1, A)
    tr_yWy = beta * tr_yy - np.sum(A * B)

    F = 0.5 * N * np.log(beta)
    F += 0.5 * np.linalg.slogdet(k_mm)[1]
    F -= 0.5 * N * np.log(np.pi)
    F -= 0.5 * np.linalg.slogdet(cov1)[1]
    F -= 0.5 * beta * psi0
    F += 0.5 * np.trace(np.linalg.solve(k_mm, psi2))
    F = (F * D - 0.5 * tr_yWy) / (N * D)

    out = F - kl_term
    result = np.asarray(out, dtype=np.float32)
    if _trace:
        return result, br
    return result


# revision 9
# speedup vs baseline: 1.2616x; 1.0243x over previous
"""Bayesian GPLVM collapsed-ELBO kernel for Trainium2 (8 NeuronCores).

Sharding: data-parallel over n (2048 rows -> 256 per core). All O(n*q)
row prep (softplus, d1/d2, w1/w2, log-sums, KL pieces) is done on host
in float64 and shipped as small per-core operand blocks; the device does
only the O(n*m) / O(n*m^2) work:

  - psi1 exponent = p1^T @ zs1 (two 33x128x64 matmuls into one PSUM
    tile), one Exp, then A += psi1^T y accumulated in PSUM (the A
    matmuls run last on PE so they don't block the psi2 train).
  - psi2: for each of 17 ij-chunks (128 upper-triangle pairs each),
    exponent = zl_chunk^T @ p2 (34x128x256, bf16), Exp on ACT, free-axis
    n-sum split between DVE and GpSimd; the last chunk's n-sum rides on
    the Exp itself via accum_out. Host sums the 8 per-core partials and
    does the small m x m linear algebra.

DMA descriptor generation is split between the gpsimd SWDGE ring
(p1z/p2 inputs, A output) and the sync HWDGE queue (zl/y inputs, psi2
stat outputs) so neither serializes the other. Only the Exp activation
table is ever needed, so there is a single ACT_TABLE_LOAD overlapping
the input DMAs.
"""

import numpy as np

N, D, Q, M = 2048, 256, 16, 64
NCORES = 8
NLOC = N // NCORES          # 256
NPAIRS = 2080               # upper-triangle pairs of 64x64
NCHUNK = 17                 # ceil(2080 / 128)
# (start_chunk, n_chunks); last chunk is accum_out on ACT
GROUPS = [(0, 4), (4, 4), (8, 4), (12, 2), (14, 2)]

_compiled = None


def _build_bass():
    import concourse.bacc as bacc
    import concourse.bass as bass  # noqa: F401
    import concourse.mybir as mybir
    from concourse.tile import TileContext

    f32 = mybir.dt.float32
    f32r = mybir.dt.float32r
    bf16 = mybir.dt.bfloat16
    AF = mybir.ActivationFunctionType
    OP = mybir.AluOpType

    nc = bacc.Bacc("TRN2", target_bir_lowering=False, num_swdge_queues=2)

    p1z_d = nc.declare_dram_parameter("p1z", [33, 320], f32r, isOutput=False)
    p2_d = nc.declare_dram_parameter("p2", [34, NLOC], bf16, isOutput=False)
    y_d = nc.declare_dram_parameter("yb", [128, 512], f32r, isOutput=False)
    zl_d = nc.declare_dram_parameter("zl", [34, NCHUNK * 128], bf16,
                                     isOutput=False)
    psi2_o = nc.declare_dram_parameter("out_psi2", [128, NCHUNK], f32,
                                       isOutput=True)
    a_o = nc.declare_dram_parameter("out_A", [M, D], f32, isOutput=True)

    with TileContext(nc) as tc:
        with (
            tc.tile_pool(name="const", bufs=1) as cpool,
            tc.tile_pool(name="scr", bufs=2) as spool,
            tc.tile_pool(name="psum", bufs=2, space="PSUM") as ppool,
            tc.tile_pool(name="psume", bufs=1, space="PSUM") as ppool_e,
            tc.tile_pool(name="psuma", bufs=1, space="PSUM") as ppool_a,
        ):
            p1z = cpool.tile([33, 320], f32r)
            p2 = cpool.tile([34, NLOC], bf16)
            yb = cpool.tile([128, 512], f32r)
            zl = cpool.tile([34, NCHUNK * 128], bf16)
            stats = cpool.tile([128, NCHUNK], f32)
            psi1c = cpool.tile([128, 128], f32r)
            a_sb = cpool.tile([M, D], f32)

            # inputs: psi1-side blocks + y on the gpsimd SWDGE ring,
            # the psi2 stream (zl) on the sync HWDGE queue
            nc.gpsimd.dma_start(out=p2[:, :], in_=p2_d[:, :])
            nc.gpsimd.dma_start(out=p1z[:, :], in_=p1z_d[:, :])
            nc.gpsimd.dma_start(out=yb[:, :], in_=y_d[:, :])
            nc.sync.dma_start(out=zl[:, 0:768], in_=zl_d[:, 0:768])
            nc.sync.dma_start(out=zl[:, 768:1792], in_=zl_d[:, 768:1792])
            nc.sync.dma_start(out=zl[:, 1792:2176], in_=zl_d[:, 1792:2176])

            # psi1 exponent for both 128-row chunks into one PSUM tile
            e1 = ppool_e.tile([128, 128], f32)
            for c in range(2):
                nc.tensor.matmul(e1[:, c * 64:(c + 1) * 64],
                                 lhsT=p1z[:, c * 128:(c + 1) * 128],
                                 rhs=p1z[:, 256:320],
                                 start=True, stop=True)
            nc.scalar.activation(psi1c[:, :], e1[:, :], AF.Exp)

            # psi2 exponent matmul train (PE) + grouped Exp (ACT) +
            # n-sum (DVE); A = psi1^T y slots in after chunk 4 so its
            # PSUM->SBUF copy lands in DVE's idle window before the
            # reduces start
            apsum = ppool_a.tile([M, D], f32)

            def a_matmuls():
                for c in range(2):
                    nc.tensor.matmul(apsum[:, :],
                                     lhsT=psi1c[:, c * 64:(c + 1) * 64],
                                     rhs=yb[:, c * 256:(c + 1) * 256],
                                     start=(c == 0), stop=(c == 1))

            ptiles = []
            for ch0, nch in GROUPS:
                p2p = ppool.tile([128, 4 * NLOC], f32, tag="p2p")
                ptiles.append((p2p, ch0, nch))
                for j in range(nch):
                    ch = ch0 + j
                    nc.tensor.matmul(
                        p2p[:, j * NLOC:(j + 1) * NLOC],
                        lhsT=zl[:, ch * 128:(ch + 1) * 128],
                        rhs=p2[:, :],
                        start=True, stop=True)
                    if ch == 4:
                        a_matmuls()
            elast = ppool_e.tile([128, NLOC], f32)
            nc.tensor.matmul(elast[:, :], lhsT=zl[:, 16 * 128:17 * 128],
                             rhs=p2[:, :], start=True, stop=True)

            nc.vector.tensor_copy(a_sb[:, :], apsum[:, :])
            nc.gpsimd.dma_start(out=a_o[:, :], in_=a_sb[:, :])

            for p2p, ch0, nch in ptiles:
                scr = spool.tile([128, 4 * NLOC], f32, tag="scr")
                w = nch * NLOC
                nc.scalar.activation(scr[:, :w], p2p[:, :w], AF.Exp)
                nc.vector.tensor_reduce(
                    stats[:, ch0:ch0 + nch],
                    scr[:, :w].rearrange("p (a b) -> p a b", b=NLOC),
                    axis=mybir.AxisListType.X, op=OP.add)
                nc.sync.dma_start(out=psi2_o[:, ch0:ch0 + nch],
                                  in_=stats[:, ch0:ch0 + nch])

            # last chunk: n-sum fused into the Exp via accum_out
            scrl = spool.tile([128, NLOC], f32)
            nc.scalar.activation(scrl[:, :], elast[:, :], AF.Exp,
                                 accum_out=stats[:, 16:17])
            nc.sync.dma_start(out=psi2_o[:, 16:17], in_=stats[:, 16:17])

    nc.compile()
    return nc


def _get_compiled():
    global _compiled
    if _compiled is None:
        _compiled = _build_bass()
    return _compiled


def kernel(y, q_mu, q_log_sigma, z, noise_raw, alpha, variance, _trace=False):
    import ml_dtypes
    from concourse.bass_utils import run_bass_kernel_spmd

    nc = _get_compiled()

    f8 = np.float64
    qm = q_mu.astype(f8)
    qls = q_log_sigma.astype(f8)
    z64 = z.astype(f8)
    al = alpha.astype(f8)
    var = f8(variance[0])
    logvar = np.log(var)

    # ---- host row prep (O(n*q)) ----
    qsig = np.logaddexp(qls, 0.0)                           # softplus
    d1 = qsig * al + 1.0
    d2 = 2.0 * al * qsig + 1.0
    w1 = al / d1
    w2 = al / d2
    lse1 = np.sum(np.log(d1), axis=1)                       # (n,)
    lse2 = np.sum(np.log(d2), axis=1)
    rt1 = np.sum(qm * qm * w1, axis=1)
    rt2 = np.sum(qm * qm * w2, axis=1)
    h1 = 2.0 * logvar - 0.5 * (rt1 + lse1)
    g = 4.0 * logvar - rt2 - 0.5 * lse2

    kl_sum = np.sum(-np.log(qsig) + 0.5 * (qsig * qsig + qm * qm - 1.0))
    tr_yy = np.sum(y.astype(f8) ** 2)

    # ---- z-side blocks (replicated) ----
    iu, ju = np.triu_indices(M)                             # (2080,)
    Su = z64[iu] + z64[ju]                                  # (2080, q)
    sqz = (z64[:, None, :] - z64[None, :, :]) ** 2          # (m, m, q)
    s1 = 0.25 * (sqz @ al)                                  # (m, m)
    zl = np.zeros((34, NCHUNK * 128), np.float32)
    zl[0:16, :NPAIRS] = Su.T
    zl[16:32, :NPAIRS] = (-0.25 * Su * Su).T
    zl[32, :NPAIRS] = 1.0
    zl[33, :NPAIRS] = -s1[iu, ju]
    zl = zl.astype(ml_dtypes.bfloat16)

    zt = z64.T                                              # (q, m)

    in_maps = []
    for i in range(NCORES):
        sl = slice(i * NLOC, (i + 1) * NLOC)
        p1z = np.zeros((33, 320), np.float32)
        p1z[0:16, 0:NLOC] = (qm[sl] * w1[sl]).T
        p1z[16:32, 0:NLOC] = w1[sl].T
        p1z[32, 0:NLOC] = h1[sl]
        p1z[0:16, 256:320] = zt
        p1z[16:32, 256:320] = -0.5 * zt * zt
        p1z[32, 256:320] = 1.0

        p2 = np.empty((34, NLOC), np.float32)
        p2[0:16] = (qm[sl] * w2[sl]).T
        p2[16:32] = w2[sl].T
        p2[32] = g[sl]
        p2[33] = 1.0

        yb = np.ascontiguousarray(
            y[sl].astype(np.float32).reshape(2, 128, D)
            .transpose(1, 0, 2).reshape(128, 512))

        in_maps.append({
            "p1z": p1z,
            "p2": p2.astype(ml_dtypes.bfloat16),
            "yb": yb,
            "zl": zl,
        })

    br = run_bass_kernel_spmd(nc, in_maps, list(range(NCORES)), trace=_trace)
    res = br.results

    psi2_part = np.zeros((128, NCHUNK), f8)
    A = np.zeros((M, D), f8)
    for r in res:
        psi2_part += r["out_psi2"].astype(f8)
        A += r["out_A"].astype(f8)

    flat = psi2_part.T.reshape(NCHUNK * 128)
    psi2 = np.empty((M, M), f8)
    psi2[iu, ju] = flat[:NPAIRS]
    psi2[ju, iu] = flat[:NPAIRS]

    kl_term = kl_sum / (N * D)

    # small m x m algebra on host
    k_mm = var * np.exp(-0.5 * (sqz @ al))                  # (m, m)
    noise_var = np.logaddexp(f8(noise_raw[0]), 0.0)
    beta = 1.0 / noise_var
    psi0 = N * var

    cov1 = beta * psi2 + k_mm
    B = np.linalg.solve(cov1, A)
    tr_yWy = beta * tr_yy - np.sum(A * B)

    F = 0.5 * N * np.log(beta)
    F += 0.5 * np.linalg.slogdet(k_mm)[1]
    F -= 0.5 * N * np.log(np.pi)
    F -= 0.5 * np.linalg.slogdet(cov1)[1]
    F -= 0.5 * beta * psi0
    F += 0.5 * np.trace(np.linalg.solve(k_mm, psi2))
    F = (F * D - 0.5 * tr_yWy) / (N * D)

    out = F - kl_term
    result = np.asarray(out, dtype=np.float32)
    if _trace:
        return result, br
    return result


# revision 10
# speedup vs baseline: 1.3459x; 1.0668x over previous
"""Bayesian GPLVM collapsed-ELBO kernel for Trainium2 (8 NeuronCores).

Sharding: data-parallel over n (2048 rows -> 256 per core). All O(n*q)
row prep (softplus, d1/d2, w1/w2, log-sums, KL pieces) is done on host
in float64 and shipped as small per-core operand blocks; the device does
only the O(n*m) / O(n*m^2) work:

  - psi1 exponent = p1^T @ zs1 (two 33x128x64 matmuls into one PSUM
    tile), one Exp, then A += psi1^T y accumulated in PSUM; these slot
    into the psi2 matmul train once their inputs land.
  - psi2: for each of 17 ij-chunks (128 upper-triangle pairs each),
    exponent = zl_chunk^T @ p2 (34x128x256, bf16), Exp on ACT, free-axis
    n-sum on DVE; the last chunk's n-sum rides the Exp via accum_out.
    Host sums the 8 per-core partials and does the small m x m algebra.

DMA plan: the psi2 stream (zl, 4 slices) goes down the sync HWDGE
queue while the psi1-side blocks (p2/p1z/yb) go down the scalar HWDGE
queue, so descriptor generation is parallel and the first psi2 matmul
can start as early as possible. All outputs drain via sync. Only the
Exp table is ever needed, so there is a single ACT_TABLE_LOAD.
"""

import numpy as np

N, D, Q, M = 2048, 256, 16, 64
NCORES = 8
NLOC = N // NCORES          # 256
NPAIRS = 2080               # upper-triangle pairs of 64x64
NCHUNK = 17                 # ceil(2080 / 128)
# (start_chunk, n_chunks) per PSUM group; last chunk of the last group
# is summed via accum_out instead of a DVE reduce
GROUPS = [(0, 4), (4, 4), (8, 4), (12, 3), (15, 2)]
ZL_SLICES = [(0, 384), (384, 896), (896, 1536), (1536, 2176)]

_compiled = None


def _build_bass():
    import concourse.bacc as bacc
    import concourse.bass as bass  # noqa: F401
    import concourse.mybir as mybir
    from concourse.tile import TileContext

    f32 = mybir.dt.float32
    f32r = mybir.dt.float32r
    bf16 = mybir.dt.bfloat16
    AF = mybir.ActivationFunctionType
    OP = mybir.AluOpType

    nc = bacc.Bacc("TRN2", target_bir_lowering=False, num_swdge_queues=2)

    p1z_d = nc.declare_dram_parameter("p1z", [33, 320], f32r, isOutput=False)
    p2_d = nc.declare_dram_parameter("p2", [34, NLOC], bf16, isOutput=False)
    y_d = nc.declare_dram_parameter("yb", [128, 512], f32r, isOutput=False)
    zl_d = nc.declare_dram_parameter("zl", [34, NCHUNK * 128], bf16,
                                     isOutput=False)
    psi2_o = nc.declare_dram_parameter("out_psi2", [128, NCHUNK], f32,
                                       isOutput=True)
    a_o = nc.declare_dram_parameter("out_A", [M, D], f32, isOutput=True)

    with TileContext(nc) as tc:
        with (
            tc.tile_pool(name="const", bufs=1) as cpool,
            tc.tile_pool(name="scr", bufs=2) as spool,
            tc.tile_pool(name="psum", bufs=3, space="PSUM") as ppool,
            tc.tile_pool(name="psume", bufs=1, space="PSUM") as ppool_e,
            tc.tile_pool(name="psuma", bufs=1, space="PSUM") as ppool_a,
        ):
            p1z = cpool.tile([33, 320], f32r)
            p2 = cpool.tile([34, NLOC], bf16)
            yb = cpool.tile([128, 512], f32r)
            zl = cpool.tile([34, NCHUNK * 128], bf16)
            stats = cpool.tile([128, NCHUNK], f32)
            psi1c = cpool.tile([128, 128], f32r)
            a_sb = cpool.tile([M, D], f32)

            # psi2 stream on the sync HWDGE queue, psi1-side blocks on
            # the scalar HWDGE queue (parallel descriptor generation)
            for c0, c1 in ZL_SLICES:
                nc.sync.dma_start(out=zl[:, c0:c1], in_=zl_d[:, c0:c1])
            nc.scalar.dma_start(out=p2[:, :], in_=p2_d[:, :])
            nc.scalar.dma_start(out=p1z[:, :], in_=p1z_d[:, :])
            nc.scalar.dma_start(out=yb[:, :], in_=y_d[:, :])

            e1 = ppool_e.tile([128, 128], f32)
            apsum = ppool_a.tile([M, D], f32)

            def e1_matmuls():
                for c in range(2):
                    nc.tensor.matmul(e1[:, c * 64:(c + 1) * 64],
                                     lhsT=p1z[:, c * 128:(c + 1) * 128],
                                     rhs=p1z[:, 256:320],
                                     start=True, stop=True)

            def a_matmuls():
                for c in range(2):
                    nc.tensor.matmul(apsum[:, :],
                                     lhsT=psi1c[:, c * 64:(c + 1) * 64],
                                     rhs=yb[:, c * 256:(c + 1) * 256],
                                     start=(c == 0), stop=(c == 1))

            # psi2 exponent matmul train; e1 after chunk 4, A after
            # chunk 8 (by then p1z/psi1c/yb have landed)
            ptiles = []
            for ch0, nch in GROUPS:
                p2p = ppool.tile([128, 4 * NLOC], f32, tag="p2p")
                ptiles.append((p2p, ch0, nch))
                for j in range(nch):
                    ch = ch0 + j
                    nc.tensor.matmul(
                        p2p[:, j * NLOC:(j + 1) * NLOC],
                        lhsT=zl[:, ch * 128:(ch + 1) * 128],
                        rhs=p2[:, :],
                        start=True, stop=True)
                    if ch == 4:
                        e1_matmuls()
                    elif ch == 8:
                        a_matmuls()

            # ACT chain: e0, psi1-exp, then remaining group exps; the
            # final chunk's exp carries accum_out (its n-sum)
            exps = []
            for gi, (p2p, ch0, nch) in enumerate(ptiles):
                scr = spool.tile([128, 4 * NLOC], f32, tag="scr")
                exps.append((p2p, scr, ch0, nch))

            def do_group(gi, last_accum=False):
                p2p, scr, ch0, nch = exps[gi]
                nred = nch - 1 if last_accum else nch
                if nred:
                    w0, w1 = 0, nred * NLOC
                    nc.scalar.activation(scr[:, w0:w1], p2p[:, w0:w1],
                                         AF.Exp)
                if last_accum:
                    w0, w1 = nred * NLOC, nch * NLOC
                    nc.scalar.activation(scr[:, w0:w1], p2p[:, w0:w1],
                                         AF.Exp,
                                         accum_out=stats[:, ch0 + nred:
                                                         ch0 + nred + 1])
                return nred

            def do_reduce(gi, nred):
                p2p, scr, ch0, nch = exps[gi]
                nc.vector.tensor_reduce(
                    stats[:, ch0:ch0 + nred],
                    scr[:, :nred * NLOC].rearrange("p (a b) -> p a b",
                                                   b=NLOC),
                    axis=mybir.AxisListType.X, op=OP.add)

            # group 0 exp, then psi1 exp, then the rest
            do_group(0)
            nc.scalar.activation(psi1c[:, :], e1[:, :], AF.Exp)
            do_reduce(0, 4)
            nc.sync.dma_start(out=psi2_o[:, 0:4], in_=stats[:, 0:4])

            do_group(1)
            do_reduce(1, 4)
            # A copy on DVE between reduces; its DMA rides sync between
            # the stat outputs
            nc.vector.tensor_copy(a_sb[:, :], apsum[:, :])
            nc.sync.dma_start(out=psi2_o[:, 4:8], in_=stats[:, 4:8])
            nc.sync.dma_start(out=a_o[:, :], in_=a_sb[:, :])

            do_group(2)
            do_reduce(2, 4)
            nc.sync.dma_start(out=psi2_o[:, 8:12], in_=stats[:, 8:12])

            do_group(3)
            do_reduce(3, 3)
            nc.sync.dma_start(out=psi2_o[:, 12:15], in_=stats[:, 12:15])

            do_group(4, last_accum=True)
            do_reduce(4, 1)
            nc.sync.dma_start(out=psi2_o[:, 15:17], in_=stats[:, 15:17])

    nc.compile()
    return nc


def _get_compiled():
    global _compiled
    if _compiled is None:
        _compiled = _build_bass()
    return _compiled


def kernel(y, q_mu, q_log_sigma, z, noise_raw, alpha, variance, _trace=False):
    import ml_dtypes
    from concourse.bass_utils import run_bass_kernel_spmd

    nc = _get_compiled()

    f8 = np.float64
    qm = q_mu.astype(f8)
    qls = q_log_sigma.astype(f8)
    z64 = z.astype(f8)
    al = alpha.astype(f8)
    var = f8(variance[0])
    logvar = np.log(var)

    # ---- host row prep (O(n*q)) ----
    qsig = np.logaddexp(qls, 0.0)                           # softplus
    d1 = qsig * al + 1.0
    d2 = 2.0 * al * qsig + 1.0
    w1 = al / d1
    w2 = al / d2
    lse1 = np.sum(np.log(d1), axis=1)                       # (n,)
    lse2 = np.sum(np.log(d2), axis=1)
    rt1 = np.sum(qm * qm * w1, axis=1)
    rt2 = np.sum(qm * qm * w2, axis=1)
    h1 = 2.0 * logvar - 0.5 * (rt1 + lse1)
    g = 4.0 * logvar - rt2 - 0.5 * lse2

    kl_sum = np.sum(-np.log(qsig) + 0.5 * (qsig * qsig + qm * qm - 1.0))
    tr_yy = np.sum(y.astype(f8) ** 2)

    # ---- z-side blocks (replicated) ----
    iu, ju = np.triu_indices(M)                             # (2080,)
    Su = z64[iu] + z64[ju]                                  # (2080, q)
    sqz = (z64[:, None, :] - z64[None, :, :]) ** 2          # (m, m, q)
    s1 = 0.25 * (sqz @ al)                                  # (m, m)
    zl = np.zeros((34, NCHUNK * 128), np.float32)
    zl[0:16, :NPAIRS] = Su.T
    zl[16:32, :NPAIRS] = (-0.25 * Su * Su).T
    zl[32, :NPAIRS] = 1.0
    zl[33, :NPAIRS] = -s1[iu, ju]
    zl = zl.astype(ml_dtypes.bfloat16)

    zt = z64.T                                              # (q, m)

    in_maps = []
    for i in range(NCORES):
        sl = slice(i * NLOC, (i + 1) * NLOC)
        p1z = np.zeros((33, 320), np.float32)
        p1z[0:16, 0:NLOC] = (qm[sl] * w1[sl]).T
        p1z[16:32, 0:NLOC] = w1[sl].T
        p1z[32, 0:NLOC] = h1[sl]
        p1z[0:16, 256:320] = zt
        p1z[16:32, 256:320] = -0.5 * zt * zt
        p1z[32, 256:320] = 1.0

        p2 = np.empty((34, NLOC), np.float32)
        p2[0:16] = (qm[sl] * w2[sl]).T
        p2[16:32] = w2[sl].T
        p2[32] = g[sl]
        p2[33] = 1.0

        yb = np.ascontiguousarray(
            y[sl].astype(np.float32).reshape(2, 128, D)
            .transpose(1, 0, 2).reshape(128, 512))

        in_maps.append({
            "p1z": p1z,
            "p2": p2.astype(ml_dtypes.bfloat16),
            "yb": yb,
            "zl": zl,
        })

    br = run_bass_kernel_spmd(nc, in_maps, list(range(NCORES)), trace=_trace)
    res = br.results

    psi2_part = np.zeros((128, NCHUNK), f8)
    A = np.zeros((M, D), f8)
    for r in res:
        psi2_part += r["out_psi2"].astype(f8)
        A += r["out_A"].astype(f8)

    flat = psi2_part.T.reshape(NCHUNK * 128)
    psi2 = np.empty((M, M), f8)
    psi2[iu, ju] = flat[:NPAIRS]
    psi2[ju, iu] = flat[:NPAIRS]

    kl_term = kl_sum / (N * D)

    # small m x m algebra on host
    k_mm = var * np.exp(-0.5 * (sqz @ al))                  # (m, m)
    noise_var = np.logaddexp(f8(noise_raw[0]), 0.0)
    beta = 1.0 / noise_var
    psi0 = N * var

    cov1 = beta * psi2 + k_mm
    B = np.linalg.solve(cov1, A)
    tr_yWy = beta * tr_yy - np.sum(A * B)

    F = 0.5 * N * np.log(beta)
    F += 0.5 * np.linalg.slogdet(k_mm)[1]
    F -= 0.5 * N * np.log(np.pi)
    F -= 0.5 * np.linalg.slogdet(cov1)[1]
    F -= 0.5 * beta * psi0
    F += 0.5 * np.trace(np.linalg.solve(k_mm, psi2))
    F = (F * D - 0.5 * tr_yWy) / (N * D)

    out = F - kl_term
    result = np.asarray(out, dtype=np.float32)
    if _trace:
        return result, br
    return result
